# revision 3
# baseline (speedup 1.0000x reference)
"""BatchGRU Trainium2 kernel v2: software-pipelined bidirectional GRU.

Layout (per core, 128 graphs):
  - x_pad DRAM [301, 64, 128]  (feature-major padded input; row 300 = ones;
    fill = -60000 so segment-max and relu(x+bias) are exact at padding)
  - per direction d: w_h_d [301, 900] = [w_hh.T ; bias rows], w_x_d likewise
  - out DRAM [64, 128, 600]  (t, graph, feat; cols 0:300 fwd, 300:600 bwd)

Per-step structure (both dirs interleaved at each phase):
  A  hMMs(s)          PE   gate preacts += hT(s) @ w_h       (Pr,Pz,Ph banks)
  B  sigmoid r,z      ACT  rz tile <- Pr,Pz
  C  xMMs(s+1) r,z    PE   next step's x-side preacts (start groups)
  D  t1 = r*hn        DVE  | u = z*h                          Pool
  E  t2 = t1+xn       DVE
  F  n = tanh(t2)     ACT
  G  w=(z-1)*n, h'=u-w, memset ones                           Pool
  H  out DMA          SP
  I  transpose h' -> T-bank staging (3 chunks)                PE
  J  single fused copy T[:,0:384] -> hT_all [128,3,128] f16   DVE
  K  xMMs(s+1) n-gate into T bank                             PE

PSUM: per dir Pr,Pz,Ph + T = 4 banks (8 total). T bank time-shares
xn accumulation (cols 0:300, consumed by t2) with transpose staging
(cols 0:384, consumed by the fused copy) within each step.
"""

import numpy as np
from contextlib import ExitStack, nullcontext

H = 300
HP = 301
LMAX = 64
BG = 128          # graphs per core
G3 = 900
NCORES = 8
KC = [(0, 128), (128, 256), (256, 301)]   # feature chunks (incl ones row)
NEG_FILL = -60000.0   # fp16-safe; relu(-60000+b)=0, never wins a max
T_OFF = [0, 128, 256]  # transpose staging offsets inside the T bank

# relu slabs for msg cols 8:56, ordered so the earliest-consumed t-slices
# (high t for the bwd dir, low t for fwd) are relu'd first
RELU_SLABS = [(48, 56), (8, 16), (40, 48), (16, 24), (32, 40), (24, 32)]


def build_gru(repeats=1, loop_repeats=1, break_chain=False, h16=False):
    import concourse.bacc as bacc
    import concourse.bass as bass
    import concourse.tile as tile
    from concourse import mybir
    from concourse.masks import make_identity

    f32 = mybir.dt.float32
    f16 = mybir.dt.float16
    AF = mybir.ActivationFunctionType
    ALU = mybir.AluOpType

    nc = bacc.Bacc()
    x_pad = nc.dram_tensor("x_pad", [HP, LMAX, BG], f16, kind="ExternalInput")
    w_h = [nc.dram_tensor(f"w_h_{d}", [HP, G3], f16, kind="ExternalInput") for d in "fb"]
    w_x = [nc.dram_tensor(f"w_x_{d}", [HP, G3], f16, kind="ExternalInput") for d in "fb"]
    fbias = nc.dram_tensor("fbias", [HP, 1], f32, kind="ExternalInput")
    out = nc.dram_tensor("out", [LMAX, BG, 2 * H], f32, kind="ExternalOutput")

    with tile.TileContext(nc) as tc, ExitStack() as ctx:
        const = ctx.enter_context(tc.tile_pool(name="const", bufs=1))
        tmp = ctx.enter_context(tc.tile_pool(name="tmp", bufs=1))
        hpool = [ctx.enter_context(tc.tile_pool(name=f"h_{d}", bufs=3)) for d in "fb"]
        htp = [ctx.enter_context(tc.tile_pool(name=f"ht_{d}", bufs=2)) for d in "fb"]
        gp = [ctx.enter_context(tc.tile_pool(name=f"g_{d}", bufs=2)) for d in "fb"]
        pp = [ctx.enter_context(tc.tile_pool(name=f"ps_{d}", bufs=1, space="PSUM"))
              for d in "fb"]
        tpp = [ctx.enter_context(tc.tile_pool(name=f"tp_{d}", bufs=1, space="PSUM"))
               for d in "fb"]

        loop_cm = tc.For_i(0, loop_repeats, 1) if loop_repeats > 1 else nullcontext()
        with loop_cm:
          for _rep in range(repeats):
            ident = const.tile([128, 128], f32, tag="ident", name="ident")
            make_identity(nc, ident)
            ident16 = const.tile([128, 128], f16, tag="ident16", name="ident16")
            make_identity(nc, ident16)

            # ---- input/weight DMAs, spread across the two HWDGE queues ----
            msg = [None] * 3
            fb = [None] * 3
            for k, (c0, c1) in enumerate(KC):
                p = c1 - c0
                msg[k] = const.tile([p, LMAX, BG], f16, tag=f"msg{k}", name=f"msg{k}")
                nc.sync.dma_start(out=msg[k], in_=x_pad[c0:c1, :, :])
                fb[k] = const.tile([p, 1], f32, tag=f"fb{k}", name=f"fb{k}")
                nc.sync.dma_start(out=fb[k], in_=fbias[c0:c1, :])
            wht = [[None] * 3 for _ in range(2)]
            wxt = [[None] * 3 for _ in range(2)]
            for d in range(2):
                for k, (c0, c1) in enumerate(KC):
                    p = c1 - c0
                    wht[d][k] = const.tile([p, G3], f16, tag=f"wh{d}{k}", name=f"wh{d}{k}")
                    nc.scalar.dma_start(out=wht[d][k], in_=w_h[d][c0:c1, :])
                    wxt[d][k] = const.tile([p, G3], f16, tag=f"wx{d}{k}", name=f"wx{d}{k}")
                    nc.scalar.dma_start(out=wxt[d][k], in_=w_x[d][c0:c1, :])

            # ---- prologue: segment max (h0T), edge relu slices ----
            h0T = [None] * 3
            for k, (c0, c1) in enumerate(KC):
                p = c1 - c0
                m1 = tmp.tile([128, 32, BG], f16, tag=f"m1{k}", name=f"m1{k}")
                nc.vector.tensor_max(m1[:p, :, :], msg[k][:, 0:32, :], msg[k][:, 32:64, :])
                w = 16
                while w >= 1:
                    nc.vector.tensor_max(
                        m1[:p, 0:w, :], m1[:p, 0:w, :], m1[:p, w: 2 * w, :]
                    )
                    w //= 2
                h0T[k] = const.tile([p, BG], f16, tag=f"h0T{k}", name=f"h0T{k}")
                nc.vector.tensor_copy(out=h0T[k], in_=m1[:p, 0, :])

                # relu on the t-slices the first scan steps consume
                nc.scalar.activation(out=msg[k][:, 0:8, :],
                                     in_=msg[k][:, 0:8, :],
                                     func=AF.Relu, bias=fb[k])
                nc.vector.tensor_scalar(
                    out=msg[k][:, 56:64, :], in0=msg[k][:, 56:64, :],
                    scalar1=fb[k], scalar2=0.0,
                    op0=ALU.add, op1=ALU.max)

            # ---- PSUM gate tiles (bufs=1 -> same banks every step) ----
            Prs, Pzs, Phs, Tts = {}, {}, {}, {}
            for d in range(2):
                Prs[d] = pp[d].tile([128, 512], f32, tag=f"Pr{d}", name=f"Pr{d}")
                Pzs[d] = pp[d].tile([128, 512], f32, tag=f"Pz{d}", name=f"Pz{d}")
                Phs[d] = pp[d].tile([128, 512], f32, tag=f"Ph{d}", name=f"Ph{d}")
                Tts[d] = tpp[d].tile([128, 512], f32, tag=f"T{d}", name=f"T{d}")
                # init the T bank once: staging cols in partitions 45:128 are
                # never fully written by the 45-wide k2 transpose, but the
                # fused copy reads the whole [128, 0:384] region
                nc.vector.memset(Tts[d][:, 0:512], 0.0)

            # ---- h0 (graph-major) via PE transpose of h0T; both dirs ----
            h_cur = [None, None]
            hT_cur = [None, None]   # list of 3 APs [p,128] f16 per dir
            T0t = Tts[0]
            T16 = T0t.bitcast(f16)
            for d in range(2):
                h_cur[d] = hpool[d].tile([128, 304], f32, tag=f"h{d}", name=f"h{d}")
            for k, (c0, c1) in enumerate(KC):
                p = c1 - c0
                nc.tensor.transpose(
                    out=T16[:, 256 * k: 256 * k + p],
                    in_=h0T[k],
                    identity=ident16[0:p, 0:p],
                )
                nc.scalar.copy(out=h_cur[0][:, c0:c1], in_=T16[:, 256 * k: 256 * k + p])
                nc.vector.tensor_copy(out=h_cur[1][:, c0:c1],
                                      in_=T16[:, 256 * k: 256 * k + p])
            for d in range(2):
                nc.gpsimd.memset(h_cur[d][:, 300:304], 1.0)
                hT_cur[d] = [h0T[0], h0T[1], h0T[2]]

            def emit_x_rz(Prs, Pzs, t_of):
                # x-side r,z gate preacts (starts the Pr/Pz accumulation groups)
                for d in range(2):
                    t = t_of[d]
                    for k in range(3):
                        lhsT = msg[k][:, t, :]
                        nc.tensor.matmul(Prs[d][:, 0:300], lhsT,
                                         wxt[d][k][:, 0:300],
                                         start=(k == 0), stop=False)
                        nc.tensor.matmul(Pzs[d][:, 0:300], lhsT,
                                         wxt[d][k][:, 300:600],
                                         start=(k == 0), stop=False)

            def emit_x_n(Tts, t_of):
                # x-side n-gate preact into the T bank
                for d in range(2):
                    t = t_of[d]
                    for k in range(3):
                        nc.tensor.matmul(Tts[d][:, 0:300], msg[k][:, t, :],
                                         wxt[d][k][:, 600:900],
                                         start=(k == 0), stop=(k == 2))

            # step 0's x-side preacts
            t0_of = {0: 0, 1: LMAX - 1}
            emit_x_rz(Prs, Pzs, t0_of)
            emit_x_n(Tts, t0_of)

            # ---- main scan ----
            for s in range(LMAX):
                ts = {0: s, 1: LMAX - 1 - s}
                tn = {0: s + 1, 1: LMAX - 2 - s}
                rzs, t1s, t2s, ngs, us, ws, hns = {}, {}, {}, {}, {}, {}, {}

                # A: h-side GEMMs, gate-major; Pr first (sigmoid r is the
                # longest chain head), then Ph (t1 input), then Pz
                for gate, (dst, lo) in enumerate(
                        ((Prs, 0), (Phs, 600), (Pzs, 300))):
                    for d in range(2):
                        for k in range(3):
                            nc.tensor.matmul(
                                dst[d][:, 0:300], hT_cur[d][k],
                                wht[d][k][:, lo:lo + 300],
                                start=(gate == 1 and k == 0),
                                stop=(k == 2))

                # B: sigmoids
                for d in range(2):
                    rzs[d] = gp[d].tile([128, 2, 300], f32, tag=f"rz{d}", name=f"rz{d}")
                    nc.scalar.activation(out=rzs[d][:, 0, :], in_=Prs[d][:, 0:300],
                                         func=AF.Sigmoid)
                for d in range(2):
                    nc.scalar.activation(out=rzs[d][:, 1, :], in_=Pzs[d][:, 0:300],
                                         func=AF.Sigmoid)

                # C: next step's x-side r,z preacts (fills PE during the tail)
                if s < LMAX - 1:
                    emit_x_rz(Prs, Pzs, tn)

                # D..G: t1 = r*hn, t2 = t1+xn (DVE, per-dir streams to avoid
                # head-of-line blocking); n = tanh(t2) (ACT); u = z*h,
                # w = (z-1)*n, h' = u-w (Pool, chunked along transpose-chunk
                # boundaries -- Pool ops have no fixed cost, and the k0
                # transpose can start as soon as cols 0:128 of h' are done).
                for d in range(2):
                    t1s[d] = gp[d].tile([128, 300], f32, tag=f"t1{d}", name=f"t1{d}")
                    t2s[d] = gp[d].tile([128, 300], f32, tag=f"t2{d}", name=f"t2{d}")
                    ngs[d] = gp[d].tile([128, 300], f32, tag=f"n{d}", name=f"n{d}")
                    ws[d] = gp[d].tile([128, 300], f32, tag=f"w{d}", name=f"w{d}")
                    us[d] = gp[d].tile([128, 300], f32, tag=f"u{d}", name=f"u{d}")
                    hns[d] = hpool[d].tile([128, 304], f32, tag=f"h{d}", name=f"h{d}")
                for d in range(2):
                    nc.vector.tensor_mul(t1s[d], rzs[d][:, 0, :], Phs[d][:, 0:300])
                    nc.vector.tensor_add(t2s[d], t1s[d], Tts[d][:, 0:300])
                for d in range(2):
                    nc.scalar.activation(out=ngs[d], in_=t2s[d], func=AF.Tanh)
                # h' = u + n - z*n  (= (1-z)*n + z*h); TensorTensor only --
                # GpSimd has no scalar_tensor_tensor on hardware
                for d in range(2):
                    nc.gpsimd.memset(hns[d][:, 300:304], 1.0)
                    nc.gpsimd.tensor_mul(us[d], rzs[d][:, 1, :], h_cur[d][:, 0:300])
                    for (a, b) in ((0, 128), (128, 256), (256, 300)):
                        nc.gpsimd.tensor_mul(ws[d][:, a:b], rzs[d][:, 1, a:b],
                                             ngs[d][:, a:b])
                        nc.gpsimd.tensor_add(hns[d][:, a:b], us[d][:, a:b],
                                             ngs[d][:, a:b])
                        nc.gpsimd.tensor_sub(hns[d][:, a:b], hns[d][:, a:b],
                                             ws[d][:, a:b])

                # H: stream the step's output
                for d in range(2):
                    nc.sync.dma_start(
                        out=out[ts[d], :, d * H: (d + 1) * H],
                        in_=hns[d][:, 0:300])

                # I/J/K: transpose h' for the next step + n-gate x preacts
                if s < LMAX - 1:
                    hTn = {}
                    for d in range(2):
                        hTn[d] = htp[d].tile([128, 3, 128], f16, tag=f"hT{d}",
                                             name=f"hT{d}")
                    for d in range(2):
                        for k, (c0, c1) in enumerate(KC):
                            p = c1 - c0
                            nc.tensor.transpose(
                                out=Tts[d][0:p, T_OFF[k]: T_OFF[k] + 128],
                                in_=hns[d][:, c0:c1],
                                identity=ident,
                            )
                    for k in range(3):
                        p = KC[k][1] - KC[k][0]
                        for d in range(2):
                            nc.vector.tensor_copy(
                                out=hTn[d][0:p, k, :],
                                in_=Tts[d][0:p, T_OFF[k]: T_OFF[k] + 128])
                    emit_x_n(Tts, tn)
                    if not break_chain:
                        for d in range(2):
                            hT_cur[d] = [hTn[d][0:(c1 - c0), k, :]
                                         for k, (c0, c1) in enumerate(KC)]
                for d in range(2):
                    h_cur[d] = hns[d]

                # trickle the middle relu slabs through spare ACT/DVE slots
                if s < len(RELU_SLABS):
                    ta, tb = RELU_SLABS[s]
                    for k in range(3):
                        if s % 2 == 0:
                            nc.vector.tensor_scalar(
                                out=msg[k][:, ta:tb, :], in0=msg[k][:, ta:tb, :],
                                scalar1=fb[k], scalar2=0.0,
                                op0=ALU.add, op1=ALU.max)
                        else:
                            nc.scalar.activation(out=msg[k][:, ta:tb, :],
                                                 in_=msg[k][:, ta:tb, :],
                                                 func=AF.Relu, bias=fb[k])

    return nc


# ---------------- host side ----------------

def prep_inputs(node, batch, pos, bias, w_ih_f, w_hh_f, b_ih_f, b_hh_f,
                w_ih_b, w_hh_b, b_ih_b, b_hh_b):
    """Build per-core in_maps for the bass kernel."""
    node = np.ascontiguousarray(np.asarray(node, dtype=np.float32))
    batch = np.asarray(batch, dtype=np.int64)
    pos = np.asarray(pos, dtype=np.int64)

    x_pad_all = np.full((HP, NCORES * LMAX * BG), NEG_FILL, dtype=np.float16)
    x_pad_all = x_pad_all.reshape(HP, NCORES, LMAX, BG)
    x_pad_all[H, :, :, :] = 1.0
    core = batch // BG
    g_loc = batch % BG
    x_pad_all[0:H, core, pos, g_loc] = node.T.astype(np.float16)

    def wset(w_ih, w_hh, b_ih, b_hh):
        w_h_aug = np.zeros((HP, G3), dtype=np.float32)
        w_h_aug[0:H, :] = np.asarray(w_hh, np.float32).T
        bh = np.asarray(b_hh, np.float32)
        bi = np.asarray(b_ih, np.float32)
        w_h_aug[H, 0:600] = bi[0:600] + bh[0:600]
        w_h_aug[H, 600:900] = bh[600:900]
        w_x_aug = np.zeros((HP, G3), dtype=np.float32)
        w_x_aug[0:H, :] = np.asarray(w_ih, np.float32).T
        w_x_aug[H, 600:900] = bi[600:900]
        return w_h_aug.astype(np.float16), w_x_aug.astype(np.float16)

    w_h_f_aug, w_x_f_aug = wset(w_ih_f, w_hh_f, b_ih_f, b_hh_f)
    w_h_b_aug, w_x_b_aug = wset(w_ih_b, w_hh_b, b_ih_b, b_hh_b)
    fbv = np.zeros((HP, 1), dtype=np.float32)
    fbv[0:H, 0] = np.asarray(bias, np.float32)

    in_maps = []
    for c in range(NCORES):
        in_maps.append({
            "x_pad": np.ascontiguousarray(x_pad_all[:, c]),
            "w_h_f": w_h_f_aug, "w_x_f": w_x_f_aug,
            "w_h_b": w_h_b_aug, "w_x_b": w_x_b_aug,
            "fbias": fbv,
        })
    return in_maps, core, g_loc, pos


def gather_output(results, core, g_loc, pos):
    """results: list of per-core {'out': [64,128,600]} -> [N, 600]"""
    outs = np.stack([np.asarray(r["out"]) for r in results])  # [8, 64, 128, 600]
    return outs[core, pos, g_loc, :]


# ---------------- entry point ----------------

_CACHE = {}


def _get_nc():
    if "nc" not in _CACHE:
        nc = build_gru()
        nc.finalize()
        _CACHE["nc"] = nc
    return _CACHE["nc"]


def kernel(**inputs):
    """Full-input / full-output BatchGRU kernel distributed over 8 NeuronCores."""
    from concourse.bass_utils import run_bass_kernel_spmd

    in_maps, core, g_loc, pos = prep_inputs(
        inputs["node"], inputs["batch"], inputs["pos"], inputs["bias"],
        inputs["w_ih_f"], inputs["w_hh_f"], inputs["b_ih_f"], inputs["b_hh_f"],
        inputs["w_ih_b"], inputs["w_hh_b"], inputs["b_ih_b"], inputs["b_hh_b"],
    )
    res = run_bass_kernel_spmd(_get_nc(), in_maps, core_ids=list(range(NCORES)))
    return gather_output(res.results, core, g_loc, pos).astype(np.float32)
